# revision 29
# baseline (speedup 1.0000x reference)
"""Trainium2 Bass kernel for nn_Attention_73375221285454.

Multi-head self-attention (B=4, N=2048, D=768, H=12, DH=64) with key-padding
mask, distributed over 8 NeuronCores.

Sharding (head-split, tensor-parallel): core c handles batch b = c//2 and
head half hg = c%2 (6 of 12 heads: columns hg*384.. of Wq/Wk/Wv and rows
hg*384.. of Wo). Each core computes its 6 heads' projections, attention, and
a PARTIAL output projection for the whole batch; the host sums the two
partial outputs of each pair. No K/V duplication, no collectives.

Token sort: attention is permutation-invariant over tokens, so the host
sorts each batch's tokens unmasked-first (queries and keys are the same
token set). Keys cover na = ceil(max_unmasked/128)*128 columns (~1152 of
2048 at a 50% mask; pad keys are killed by the additive exp bias); queries
are trimmed further to na_q = ceil(max_unmasked/16)*16 columns. Masked-query
rows never touch the device: the reference gives them a uniform softmax over
ALL keys, i.e. (mean_j x[b,j] @ Wv) @ Wo, computed on the host in numpy.

Device algorithm per core (all matmuls bf16, fp32 PSUM):
  qT/kT = (Wq_h.T @ xs) bf16                 [128, 3, na]
  vaug[jt][key, h, 0:64] = V, [.., 64] = 1.0 (ones column makes P@V also
                                              accumulate the softmax sum s)
  per head h, key tile jt:
    S^T[128 keys, na] = kT_h,jt.T @ qT_h     (PSUM fp32)
    P^T = exp(0.125*S^T + bias[key]) -> bf16 (ACT writes the matmul-ready
         bias: 0 active / -30000 pad key      P tile directly; no copies on
         so pad keys contribute exactly 0)    the P critical path)
  per head h, query chunk qc:
    psO[72, qc] += vaug_jt.T @ P^T_jt        (row 64 of psO = denominator s)
    attnT_h[:, qc] = psO[0:64] * (1/s)
  out_partial = attnT.T @ Wo_h  [na, 768] fp32

bf16 is as fast as fp32r/fp8 on the PE for these shapes (cost is one moving
column per cycle regardless of dtype or contraction depth), but halves SBUF
and lets ACT write P without a bitcast copy. fp8 attempts all failed:
quantizing q/k is amplified by sqrt(2*DH) in the logits (~0.4 sigma), fp8 V
costs ~3e-2 max-rel, and DoubleRow gives no per-column speedup on real HW.

1/s runs on DVE reciprocal_approx_fast (~18-bit) from a partition-0 staging
tile (the custom-DVE op misreads partition-offset inputs); gpsimd broadcasts
it; a DVE multiply normalizes while draining PSUM. The whole P@V + drain +
leftover projections stream is emitted as "filler" between score tiles so
the PE never idles while ACT works through the exp stream.
"""

import sys

sys.path.insert(0, "/opt/trn_rl_repo")

import numpy as np
import ml_dtypes

import concourse.bass as bass  # noqa: F401
import concourse.mybir as mybir
import concourse.tile as tile
from concourse import bacc
from concourse.bass_utils import run_bass_kernel_spmd

P = 128
B, N, D = 4, 2048, 768
H, DH = 12, 64
HPC = H // 2            # heads per core
HD = HPC * DH           # 384 projected dims per core
DC = D // P             # 6 contraction chunks
HDT = HD // P           # 3 head-dim chunks of 128
SCALE = DH ** -0.5      # 0.125
EXP_SHIFT = 0.0         # bf16 P cannot overflow; no shift needed
DHW = 72                # vhl per-head width: 6*72 B stride is 16B-aligned
                        # (DoubleRow LdWeights requires 16B-aligned steps)
MASK_NEG = -30000.0
BF16 = ml_dtypes.bfloat16
FP8 = ml_dtypes.float8_e4m3

f32 = mybir.dt.float32
bf16 = mybir.dt.bfloat16
fp8 = mybir.dt.float8e4

_BUILD_CACHE = {}


def _chunks(total, step):
    out = []
    off = 0
    while off < total:
        sz = min(step, total - off)
        out.append((off, sz))
        off += sz
    return out


def build(njt: int, na_q: int) -> "bacc.Bacc":
    """SPMD program: njt key tiles (na = njt*128 keys), na_q query columns."""
    key = (njt, na_q)
    if key in _BUILD_CACHE:
        return _BUILD_CACHE[key]

    na = njt * P
    psw = ((na * 4 + 2047) // 2048) * 512   # na rounded up to PSUM banks
    pss_bufs = 2 if 2 * psw * 4 + 2 * 2048 <= 16384 else 1

    nc = bacc.Bacc()
    xsT_d = nc.declare_dram_parameter("xsT", [D, na], bf16, isOutput=False)
    wq_d = nc.declare_dram_parameter("wq", [D, HD], bf16, isOutput=False)
    wk_d = nc.declare_dram_parameter("wk", [D, HD], bf16, isOutput=False)
    wv_d = nc.declare_dram_parameter("wv", [D, HD], bf16, isOutput=False)
    wo_d = nc.declare_dram_parameter("woT", [HD, D], bf16, isOutput=False)
    # cmneg[p, t] = EXP_SHIFT if key (t*128+p) active else MASK_NEG
    cm_d = nc.declare_dram_parameter("cmneg", [P, njt], f32, isOutput=False)
    out_d = nc.declare_dram_parameter("out", [na_q, D], f32, isOutput=True)

    xs_r = xsT_d.rearrange("(c p) n -> p c n", p=P)
    wq_r = wq_d.rearrange("(c p) e -> p c e", p=P)
    wk_r = wk_d.rearrange("(c p) e -> p c e", p=P)
    wv_r = wv_d.rearrange("(c p) e -> p c e", p=P)
    wo_r = wo_d.rearrange("(c p) e -> p c e", p=P)

    # chunk offsets must stay multiples of 512 so every matmul destination
    # sits inside a single 2KB PSUM bank
    col_ch = _chunks(na_q, 512)      # query column chunks (moving <= 512)
    key_ch = _chunks(na, 512)        # key/token column chunks (K proj)
    n_qc = len(col_ch)
    it_sizes = [P] * (na_q // P) + ([na_q % P] if na_q % P else [])

    with tile.TileContext(nc) as tc:
        with tc.tile_pool(name="persist", bufs=1) as persist:
            cmneg = persist.tile([P, njt], f32)
            xs = persist.tile([P, DC, na], bf16)
            wq_sb = persist.tile([P, DC, HD], bf16)
            wk_sb = persist.tile([P, DC, HD], bf16)
            wv_sb = persist.tile([P, DC, HD], bf16)
            wo_sb = persist.tile([P, HDT, D], bf16)
            # half-column chunks: the first Q0 groups and V-proj tiles only
            # need the low columns, so they start ~2us before the full load
            half = (na // 2 + 127) // 128 * 128
            for dc in range(DC):
                nc.gpsimd.dma_start(out=xs[:, dc, 0:half],
                                    in_=xs_r[:, dc, 0:half])
            for dc in range(DC):
                nc.gpsimd.dma_start(out=xs[:, dc, half:na],
                                    in_=xs_r[:, dc, half:na])
            nc.gpsimd.dma_start(out=wq_sb, in_=wq_r)
            nc.gpsimd.dma_start(out=wk_sb, in_=wk_r)
            nc.sync.dma_start(out=cmneg, in_=cm_d.ap())
            nc.sync.dma_start(out=wv_sb, in_=wv_r)
            nc.sync.dma_start(out=wo_sb, in_=wo_r)

            qT = persist.tile([P, HDT, na_q], bf16)
            kT = persist.tile([P, HDT, na], bf16)
            attnT = persist.tile([P, HDT, na_q], bf16)
            vaug = persist.tile([P, njt, HPC, DHW], bf16)
            pts0 = persist.tile([P, njt, na_q], bf16)
            pts1 = persist.tile([P, njt, na_q], bf16)

            # vaug: ones column 64 (softmax-sum row), zeros elsewhere
            nc.vector.memset(vaug[:, :, :, DH:DHW], 0.0)
            nc.vector.memset(vaug[:, :, :, DH : DH + 1], 1.0)

            attn_scope = nc.named_scope("main"); attn_scope.__enter__()
            psxp_cm = tc.tile_pool(name="psx", bufs=2, space="PSUM")
            psxp = psxp_cm.__enter__()
            nrm_cm = tc.tile_pool(name="nrm", bufs=3)
            nrm = nrm_cm.__enter__()
            psSp_cm = tc.tile_pool(name="psS", bufs=pss_bufs, space="PSUM")
            psSp = psSp_cm.__enter__()

            def proj_full(w_sb, dst, t, chunks):
                """Full-width projection of head-dim chunk t via the big pool."""
                ps = psSp.tile([P, psw], f32, tag="psS", name=f"proj{t}")
                for off, sz in chunks:
                    for dc in range(DC):
                        nc.tensor.matmul(
                            ps[:, off : off + sz],
                            w_sb[:, dc, t * P : (t + 1) * P],
                            xs[:, dc, off : off + sz],
                            start=(dc == 0),
                            stop=(dc == DC - 1),
                        )
                w = chunks[-1][0] + chunks[-1][1]
                nc.vector.tensor_copy(dst[:, t, 0:w], ps[:, 0:w])

            # -- PE filler work, emitted piecewise between score tiles --
            filler = []

            def f_vproj(jt):
                def emit():
                    psv = psxp.tile([P, 512], f32, tag="psx", name=f"psv{jt}")
                    for dc in range(DC):
                        nc.tensor.matmul(
                            psv[:, 0:HD],
                            xs[:, dc, jt * P : (jt + 1) * P],
                            wv_sb[:, dc, :],
                            start=(dc == 0),
                            stop=(dc == DC - 1),
                        )
                    nc.vector.tensor_copy(
                        vaug[:, jt, :, 0:DH],
                        psv[:, 0:HD].rearrange("p (h d) -> p h d", h=HPC),
                    )
                return emit

            def f_projchunk(w_sb, dst, t, off, sz):
                def emit():
                    ps = psxp.tile([P, 512], f32, tag="psx",
                                   name=f"pc{t}_{off}")
                    for dc in range(DC):
                        nc.tensor.matmul(
                            ps[:, 0:sz],
                            w_sb[:, dc, t * P : (t + 1) * P],
                            xs[:, dc, off : off + sz],
                            start=(dc == 0),
                            stop=(dc == DC - 1),
                        )
                    nc.vector.tensor_copy(dst[:, t, off : off + sz], ps[:, 0:sz])
                return emit

            psO_state = {}

            def f_vatp(h, i, jt):
                """One P@V accumulation step; allocates psO at jt==0 and
                drains (normalize into attnT) after the last jt. Mid-stream
                heads copy psO out first so the PSUM slot frees in ~1.6us
                (the next P@V group's allocation waits on it); the last head
                multiplies straight from PSUM because there attnT latency is
                what gates the output projection."""
                hdt, pb = h // 2, DH * (h % 2)
                pts = pts0 if h % 2 == 0 else pts1
                off, sz = col_ch[i]
                # fast release for ALL heads: since the tail emits both
                # remaining P@V chunks before any out-proj, even the last
                # head's normalize chain has slack to run off-staging
                fast_pso = True

                def emit():
                    if jt == 0:
                        psO_state[(h, i)] = psxp.tile(
                            [P, 512], f32, tag="psx", name=f"psO{h}_{i}"
                        )
                    psO = psO_state[(h, i)]
                    nc.tensor.matmul(
                        psO[0:DHW, 0:sz],
                        vaug[:, jt, h, :],
                        pts[:, jt, off : off + sz],
                        start=(jt == 0),
                        stop=(jt == njt - 1),
                    )
                    if jt == njt - 1:
                        # stage s on a fresh partition-0 tile: the custom-DVE
                        # approx reciprocal misreads partition-offset inputs
                        s_c = nrm.tile([1, 512], f32, tag="s_c")
                        nc.vector.tensor_copy(
                            s_c[:, 0:sz], psO[DH : DH + 1, 0:sz]
                        )
                        if fast_pso:
                            attnU = nrm.tile([DH, 512], bf16, tag="attnU")
                            nc.vector.tensor_copy(
                                attnU[:, 0:sz], psO[0:DH, 0:sz]
                            )
                            num = attnU
                        else:
                            num = psO
                        r_row = nrm.tile([1, 512], f32, tag="r_row")
                        nc.vector.reciprocal_approx_fast(
                            out=r_row[:, 0:sz], in_=s_c[:, 0:sz]
                        )
                        rb = nrm.tile([DH, 512], f32, tag="rb")
                        nc.gpsimd.partition_broadcast(
                            rb[:, 0:sz], r_row[:, 0:sz], channels=DH
                        )
                        nc.vector.tensor_mul(
                            attnT[pb : pb + DH, hdt, off : off + sz],
                            num[0:DH, 0:sz],
                            rb[:, 0:sz],
                        )
                        del psO_state[(h, i)]
                return emit

            # PE p-state warmup: dummy matmuls while the input DMAs are in
            # flight, so Q0/K0 run at full clock instead of the cold 0.65GHz
            warm = nrm.tile([P, 512], bf16, tag="warm")
            nc.vector.memset(warm, 0.0)
            for w in range(12):
                ps = psSp.tile([P, psw], f32, tag="psS", name=f"warm{w}")
                nc.tensor.matmul(
                    ps[:, 0:512], warm[:, 0:128], warm[:, 0:512],
                    start=True, stop=True,
                )

            # head-dim chunk 0 of Q and K up front; the rest is filler
            proj_full(wq_sb, qT, 0, col_ch)
            proj_full(wk_sb, kT, 0, key_ch)
            for jt in range(njt):
                filler.append(f_vproj(jt))
            for t in (1, 2):
                for w_sb, dst, chunks in ((wq_sb, qT, col_ch),
                                          (wk_sb, kT, key_ch)):
                    for off, sz in chunks:
                        filler.append(f_projchunk(w_sb, dst, t, off, sz))

            # ---------------- attention ----------------
            for h in range(HPC):
                hdt, pb = h // 2, DH * (h % 2)
                pts = pts0 if h % 2 == 0 else pts1
                if h > 0:   # P@V of the previous head becomes filler
                    for i in range(n_qc):
                        for jt in range(njt):
                            filler.append(f_vatp(h - 1, i, jt))
                for jt in range(njt):
                    psS = psSp.tile([P, psw], f32, tag="psS")
                    for off, sz in col_ch:
                        nc.tensor.matmul(
                            psS[:, off : off + sz],
                            kT[pb : pb + DH, hdt, jt * P : (jt + 1) * P],
                            qT[pb : pb + DH, hdt, off : off + sz],
                            start=True,
                            stop=True,
                        )
                    # drip filler to keep PE busy during the ACT-bound loop
                    budget = 2 if h == 0 else (4 if h == HPC - 1 else 3)
                    for _ in range(min(budget, len(filler))):
                        filler.pop(0)()
                    nc.scalar.activation(
                        pts[:, jt, 0:na_q],
                        psS[:, 0:na_q],
                        mybir.ActivationFunctionType.Exp,
                        bias=cmneg[:, jt : jt + 1],
                        scale=SCALE,
                    )
                    if h == HPC - 1:
                        # last head's first P@V chunk rides its own exp stream
                        filler.append(f_vatp(h, 0, jt))
            for f in filler:   # leftover filler (rare)
                f()

            psSp_cm.__exit__(None, None, None)

            # ------------- tail: last head's P@V + output projection -------
            with tc.tile_pool(name="psF", bufs=3, space="PSUM") as psFp, \
                 tc.tile_pool(name="fin", bufs=3) as fin:

                def oproj(it):
                    rows = it_sizes[it]
                    psF = psFp.tile([P, 1024], f32, tag="psF")
                    for off, sz in ((0, 512), (512, 256)):
                        for c in range(HDT):
                            nc.tensor.matmul(
                                psF[0:rows, off : off + sz],
                                attnT[:, c, it * P : it * P + rows],
                                wo_sb[:, c, off : off + sz],
                                start=(c == 0),
                                stop=(c == HDT - 1),
                            )
                    out_sb = fin.tile([P, D], f32, tag="out_sb")
                    nc.scalar.copy(out_sb[0:rows, :], psF[0:rows, 0:D])
                    # idle Pool sequencer: 25ns dispatch vs 565ns on SP
                    nc.gpsimd.dma_start(
                        out=out_d.ap()[it * P : it * P + rows, :],
                        in_=out_sb[0:rows, :],
                    )

                # both remaining P@V chunks first, then all out-proj tiles:
                # every drain chain then overlaps matmul work instead of
                # gating the final out-proj batch
                h = HPC - 1
                for i in range(1, n_qc):
                    for jt in range(njt):
                        f_vatp(h, i, jt)()
                for it in range(len(it_sizes)):
                    oproj(it)

            nrm_cm.__exit__(None, None, None)
            psxp_cm.__exit__(None, None, None)
            attn_scope.__exit__(None, None, None)

    nc.compile()
    _BUILD_CACHE[key] = nc
    return nc


def _marshal(x, x_mask, Wq, Wk, Wv, Wo):
    """Build per-core input maps. Returns (in_maps, njt, na_q, orders, counts)."""
    x = np.asarray(x, dtype=np.float32)
    x_mask = np.asarray(x_mask).astype(bool)
    Wq = np.asarray(Wq, dtype=np.float32)
    Wk = np.asarray(Wk, dtype=np.float32)
    Wv = np.asarray(Wv, dtype=np.float32)
    Wo = np.asarray(Wo, dtype=np.float32)

    orders = [np.argsort(~x_mask[b], kind="stable") for b in range(B)]
    counts = [int(x_mask[b].sum()) for b in range(B)]
    njt = max(1, -(-max(counts) // P))
    na = njt * P
    # query trimming below 512-chunk granularity fragments the PE pipeline
    # and drops the sustained PE clock (measured): keep full width
    na_q = na

    xsTs, cms = [], []
    for b in range(B):
        xs_sorted = x[b][orders[b][:na]]                 # [na, 768]
        xsTs.append(np.ascontiguousarray(xs_sorted.T.astype(BF16)))
        key_act = np.arange(na) < counts[b]
        cm = np.where(key_act, EXP_SHIFT, MASK_NEG).astype(np.float32)
        cms.append(np.ascontiguousarray(cm.reshape(njt, P).T))

    whs = []
    for hg in range(2):
        cols = slice(hg * HD, (hg + 1) * HD)
        whs.append({
            "wq": np.ascontiguousarray(Wq[:, cols].astype(BF16)),
            "wk": np.ascontiguousarray(Wk[:, cols].astype(BF16)),
            "wv": np.ascontiguousarray(Wv[:, cols].astype(BF16)),
            "woT": np.ascontiguousarray(Wo[cols, :].astype(BF16)),
        })

    in_maps = []
    for c in range(8):
        b, hg = c // 2, c % 2
        in_maps.append({
            "xsT": xsTs[b], "cmneg": cms[b], **whs[hg],
        })
    return in_maps, njt, na_q, orders, counts


def run(x, x_mask, Wq, Wk, Wv, Wo, trace=False, tmpdir=None):
    """Run on 8 cores; returns (full_output, BassKernelResults)."""
    x = np.asarray(x, dtype=np.float32)
    Wv_f = np.asarray(Wv, dtype=np.float32)
    Wo_f = np.asarray(Wo, dtype=np.float32)
    in_maps, njt, na_q, orders, counts = _marshal(x, x_mask, Wq, Wk, Wv, Wo)
    nc = build(njt, na_q)
    res = run_bass_kernel_spmd(
        nc, in_maps, core_ids=list(range(8)), trace=trace, tmpdir=tmpdir
    )
    out = np.empty((B, N, D), dtype=np.float32)
    for b in range(B):
        s = (res.results[2 * b]["out"].astype(np.float32)
             + res.results[2 * b + 1]["out"].astype(np.float32))
        nr = counts[b]
        out[b, orders[b][:nr]] = s[:nr]
        if nr < N:
            # masked queries: uniform softmax over ALL keys
            mu = x[b].astype(np.float64).mean(axis=0)
            urow = (mu @ Wv_f.astype(np.float64)) @ Wo_f.astype(np.float64)
            out[b, orders[b][nr:]] = urow.astype(np.float32)
    return out, res


def kernel(**inputs) -> np.ndarray:
    out, _ = run(
        inputs["x"], inputs["x_mask"],
        inputs["Wq"], inputs["Wk"], inputs["Wv"], inputs["Wo"],
        trace=False,
    )
    return out


# revision 30
# speedup vs baseline: 1.0043x; 1.0043x over previous
"""Trainium2 Bass kernel for nn_Attention_73375221285454.

Multi-head self-attention (B=4, N=2048, D=768, H=12, DH=64) with key-padding
mask, distributed over 8 NeuronCores.

Sharding (head-split, tensor-parallel): core c handles batch b = c//2 and
head half hg = c%2 (6 of 12 heads: columns hg*384.. of Wq/Wk/Wv and rows
hg*384.. of Wo). Each core computes its 6 heads' projections, attention, and
a PARTIAL output projection for the whole batch; the host sums the two
partial outputs of each pair. No K/V duplication, no collectives.

Token sort: attention is permutation-invariant over tokens, so the host
sorts each batch's tokens unmasked-first (queries and keys are the same
token set). Keys cover na = ceil(max_unmasked/128)*128 columns (~1152 of
2048 at a 50% mask; pad keys are killed by the additive exp bias); queries
are trimmed further to na_q = ceil(max_unmasked/16)*16 columns. Masked-query
rows never touch the device: the reference gives them a uniform softmax over
ALL keys, i.e. (mean_j x[b,j] @ Wv) @ Wo, computed on the host in numpy.

Device algorithm per core (all matmuls bf16, fp32 PSUM):
  qT/kT = (Wq_h.T @ xs) bf16                 [128, 3, na]
  vaug[jt][key, h, 0:64] = V, [.., 64] = 1.0 (ones column makes P@V also
                                              accumulate the softmax sum s)
  per head h, key tile jt:
    S^T[128 keys, na] = kT_h,jt.T @ qT_h     (PSUM fp32)
    P^T = exp(0.125*S^T + bias[key]) -> bf16 (ACT writes the matmul-ready
         bias: 0 active / -30000 pad key      P tile directly; no copies on
         so pad keys contribute exactly 0)    the P critical path)
  per head h, query chunk qc:
    psO[72, qc] += vaug_jt.T @ P^T_jt        (row 64 of psO = denominator s)
    attnT_h[:, qc] = psO[0:64] * (1/s)
  out_partial = attnT.T @ Wo_h  [na, 768] fp32

bf16 is as fast as fp32r/fp8 on the PE for these shapes (cost is one moving
column per cycle regardless of dtype or contraction depth), but halves SBUF
and lets ACT write P without a bitcast copy. fp8 attempts all failed:
quantizing q/k is amplified by sqrt(2*DH) in the logits (~0.4 sigma), fp8 V
costs ~3e-2 max-rel, and DoubleRow gives no per-column speedup on real HW.

1/s runs on DVE reciprocal_approx_fast (~18-bit) from a partition-0 staging
tile (the custom-DVE op misreads partition-offset inputs); gpsimd broadcasts
it; a DVE multiply normalizes while draining PSUM. The whole P@V + drain +
leftover projections stream is emitted as "filler" between score tiles so
the PE never idles while ACT works through the exp stream.
"""

import sys

sys.path.insert(0, "/opt/trn_rl_repo")

import numpy as np
import ml_dtypes

import concourse.bass as bass  # noqa: F401
import concourse.mybir as mybir
import concourse.tile as tile
from concourse import bacc
from concourse.bass_utils import run_bass_kernel_spmd

P = 128
B, N, D = 4, 2048, 768
H, DH = 12, 64
HPC = H // 2            # heads per core
HD = HPC * DH           # 384 projected dims per core
DC = D // P             # 6 contraction chunks
HDT = HD // P           # 3 head-dim chunks of 128
SCALE = DH ** -0.5      # 0.125
EXP_SHIFT = 0.0         # bf16 P cannot overflow; no shift needed
DHW = 72                # vhl per-head width: 6*72 B stride is 16B-aligned
                        # (DoubleRow LdWeights requires 16B-aligned steps)
MASK_NEG = -30000.0
BF16 = ml_dtypes.bfloat16
FP8 = ml_dtypes.float8_e4m3

f32 = mybir.dt.float32
bf16 = mybir.dt.bfloat16
fp8 = mybir.dt.float8e4

_BUILD_CACHE = {}


def _chunks(total, step):
    out = []
    off = 0
    while off < total:
        sz = min(step, total - off)
        out.append((off, sz))
        off += sz
    return out


def build(njt: int, na_q: int) -> "bacc.Bacc":
    """SPMD program: njt key tiles (na = njt*128 keys), na_q query columns."""
    key = (njt, na_q)
    if key in _BUILD_CACHE:
        return _BUILD_CACHE[key]

    na = njt * P
    psw = ((na * 4 + 2047) // 2048) * 512   # na rounded up to PSUM banks
    pss_bufs = 2 if 2 * psw * 4 + 2 * 2048 <= 16384 else 1

    nc = bacc.Bacc()
    xsT_d = nc.declare_dram_parameter("xsT", [D, na], bf16, isOutput=False)
    wq_d = nc.declare_dram_parameter("wq", [D, HD], bf16, isOutput=False)
    wk_d = nc.declare_dram_parameter("wk", [D, HD], bf16, isOutput=False)
    wv_d = nc.declare_dram_parameter("wv", [D, HD], bf16, isOutput=False)
    wo_d = nc.declare_dram_parameter("woT", [HD, D], bf16, isOutput=False)
    # cmneg[p, t] = EXP_SHIFT if key (t*128+p) active else MASK_NEG
    cm_d = nc.declare_dram_parameter("cmneg", [P, njt], f32, isOutput=False)
    out_d = nc.declare_dram_parameter("out", [na_q, D], f32, isOutput=True)

    xs_r = xsT_d.rearrange("(c p) n -> p c n", p=P)
    wq_r = wq_d.rearrange("(c p) e -> p c e", p=P)
    wk_r = wk_d.rearrange("(c p) e -> p c e", p=P)
    wv_r = wv_d.rearrange("(c p) e -> p c e", p=P)
    wo_r = wo_d.rearrange("(c p) e -> p c e", p=P)

    # chunk offsets must stay multiples of 512 so every matmul destination
    # sits inside a single 2KB PSUM bank
    col_ch = _chunks(na_q, 512)      # query column chunks (moving <= 512)
    key_ch = _chunks(na, 512)        # key/token column chunks (K proj)
    n_qc = len(col_ch)
    it_sizes = [P] * (na_q // P) + ([na_q % P] if na_q % P else [])

    with tile.TileContext(nc) as tc:
        with tc.tile_pool(name="persist", bufs=1) as persist:
            cmneg = persist.tile([P, njt], f32)
            xs = persist.tile([P, DC, na], bf16)
            wq_sb = persist.tile([P, DC, HD], bf16)
            wk_sb = persist.tile([P, DC, HD], bf16)
            wv_sb = persist.tile([P, DC, HD], bf16)
            wo_sb = persist.tile([P, HDT, D], bf16)
            # half-column chunks: the first Q0 groups and V-proj tiles only
            # need the low columns, so they start ~2us before the full load
            half = (na // 2 + 127) // 128 * 128
            for dc in range(DC):
                nc.gpsimd.dma_start(out=xs[:, dc, 0:half],
                                    in_=xs_r[:, dc, 0:half])
            for dc in range(DC):
                nc.gpsimd.dma_start(out=xs[:, dc, half:na],
                                    in_=xs_r[:, dc, half:na])
            nc.gpsimd.dma_start(out=wq_sb, in_=wq_r)
            nc.gpsimd.dma_start(out=wk_sb, in_=wk_r)
            nc.sync.dma_start(out=cmneg, in_=cm_d.ap())
            nc.sync.dma_start(out=wv_sb, in_=wv_r)
            nc.sync.dma_start(out=wo_sb, in_=wo_r)

            qT = persist.tile([P, HDT, na_q], bf16)
            kT = persist.tile([P, HDT, na], bf16)
            attnT = persist.tile([P, HDT, na_q], bf16)
            vaug = persist.tile([P, njt, HPC, DHW], bf16)
            pts0 = persist.tile([P, njt, na_q], bf16)
            pts1 = persist.tile([P, njt, na_q], bf16)

            # vaug: ones column 64 (softmax-sum row), zeros elsewhere
            nc.vector.memset(vaug[:, :, :, DH:DHW], 0.0)
            nc.vector.memset(vaug[:, :, :, DH : DH + 1], 1.0)

            attn_scope = nc.named_scope("main"); attn_scope.__enter__()
            psxp_cm = tc.tile_pool(name="psx", bufs=2, space="PSUM")
            psxp = psxp_cm.__enter__()
            nrm_cm = tc.tile_pool(name="nrm", bufs=3)
            nrm = nrm_cm.__enter__()
            psSp_cm = tc.tile_pool(name="psS", bufs=pss_bufs, space="PSUM")
            psSp = psSp_cm.__enter__()

            def proj_full(w_sb, dst, t, chunks):
                """Full-width projection of head-dim chunk t via the big pool."""
                ps = psSp.tile([P, psw], f32, tag="psS", name=f"proj{t}")
                for off, sz in chunks:
                    for dc in range(DC):
                        nc.tensor.matmul(
                            ps[:, off : off + sz],
                            w_sb[:, dc, t * P : (t + 1) * P],
                            xs[:, dc, off : off + sz],
                            start=(dc == 0),
                            stop=(dc == DC - 1),
                        )
                w = chunks[-1][0] + chunks[-1][1]
                nc.vector.tensor_copy(dst[:, t, 0:w], ps[:, 0:w])

            # -- PE filler work, emitted piecewise between score tiles --
            filler = []

            def f_vproj(jt):
                def emit():
                    psv = psxp.tile([P, 512], f32, tag="psx", name=f"psv{jt}")
                    for dc in range(DC):
                        nc.tensor.matmul(
                            psv[:, 0:HD],
                            xs[:, dc, jt * P : (jt + 1) * P],
                            wv_sb[:, dc, :],
                            start=(dc == 0),
                            stop=(dc == DC - 1),
                        )
                    nc.vector.tensor_copy(
                        vaug[:, jt, :, 0:DH],
                        psv[:, 0:HD].rearrange("p (h d) -> p h d", h=HPC),
                    )
                return emit

            def f_projchunk(w_sb, dst, t, off, sz):
                def emit():
                    ps = psxp.tile([P, 512], f32, tag="psx",
                                   name=f"pc{t}_{off}")
                    for dc in range(DC):
                        nc.tensor.matmul(
                            ps[:, 0:sz],
                            w_sb[:, dc, t * P : (t + 1) * P],
                            xs[:, dc, off : off + sz],
                            start=(dc == 0),
                            stop=(dc == DC - 1),
                        )
                    nc.vector.tensor_copy(dst[:, t, off : off + sz], ps[:, 0:sz])
                return emit

            psO_state = {}

            def f_vatp(h, i, jt):
                """One P@V accumulation step; allocates psO at jt==0 and
                drains (normalize into attnT) after the last jt. Mid-stream
                heads copy psO out first so the PSUM slot frees in ~1.6us
                (the next P@V group's allocation waits on it); the last head
                multiplies straight from PSUM because there attnT latency is
                what gates the output projection."""
                hdt, pb = h // 2, DH * (h % 2)
                pts = pts0 if h % 2 == 0 else pts1
                off, sz = col_ch[i]
                # fast PSUM release matters mid-stream (next P@V group's
                # allocation waits on it) and, in the tail, only for the
                # last head's chunk 0 (psx rotation depth 2 -> chunk 2's
                # allocation waits chunk 0's release); chunks 1-2 keep the
                # direct multiply so out-proj sees attnT sooner
                fast_pso = h < HPC - 1 or i == 0

                def emit():
                    if jt == 0:
                        psO_state[(h, i)] = psxp.tile(
                            [P, 512], f32, tag="psx", name=f"psO{h}_{i}"
                        )
                    psO = psO_state[(h, i)]
                    nc.tensor.matmul(
                        psO[0:DHW, 0:sz],
                        vaug[:, jt, h, :],
                        pts[:, jt, off : off + sz],
                        start=(jt == 0),
                        stop=(jt == njt - 1),
                    )
                    if jt == njt - 1:
                        # stage s on a fresh partition-0 tile: the custom-DVE
                        # approx reciprocal misreads partition-offset inputs
                        s_c = nrm.tile([1, 512], f32, tag="s_c")
                        nc.vector.tensor_copy(
                            s_c[:, 0:sz], psO[DH : DH + 1, 0:sz]
                        )
                        if fast_pso:
                            attnU = nrm.tile([DH, 512], bf16, tag="attnU")
                            nc.vector.tensor_copy(
                                attnU[:, 0:sz], psO[0:DH, 0:sz]
                            )
                            num = attnU
                        else:
                            num = psO
                        r_row = nrm.tile([1, 512], f32, tag="r_row")
                        nc.vector.reciprocal_approx_fast(
                            out=r_row[:, 0:sz], in_=s_c[:, 0:sz]
                        )
                        rb = nrm.tile([DH, 512], f32, tag="rb")
                        nc.gpsimd.partition_broadcast(
                            rb[:, 0:sz], r_row[:, 0:sz], channels=DH
                        )
                        nc.vector.tensor_mul(
                            attnT[pb : pb + DH, hdt, off : off + sz],
                            num[0:DH, 0:sz],
                            rb[:, 0:sz],
                        )
                        del psO_state[(h, i)]
                return emit

            # PE p-state warmup: dummy matmuls while the input DMAs are in
            # flight, so Q0/K0 run at full clock instead of the cold 0.65GHz
            warm = nrm.tile([P, 512], bf16, tag="warm")
            nc.vector.memset(warm, 0.0)
            for w in range(12):
                ps = psSp.tile([P, psw], f32, tag="psS", name=f"warm{w}")
                nc.tensor.matmul(
                    ps[:, 0:512], warm[:, 0:128], warm[:, 0:512],
                    start=True, stop=True,
                )

            # head-dim chunk 0 of Q and K up front; the rest is filler
            proj_full(wq_sb, qT, 0, col_ch)
            proj_full(wk_sb, kT, 0, key_ch)
            for jt in range(njt):
                filler.append(f_vproj(jt))
            for t in (1, 2):
                for w_sb, dst, chunks in ((wq_sb, qT, col_ch),
                                          (wk_sb, kT, key_ch)):
                    for off, sz in chunks:
                        filler.append(f_projchunk(w_sb, dst, t, off, sz))

            # ---------------- attention ----------------
            for h in range(HPC):
                hdt, pb = h // 2, DH * (h % 2)
                pts = pts0 if h % 2 == 0 else pts1
                if h > 0:   # P@V of the previous head becomes filler
                    for i in range(n_qc):
                        for jt in range(njt):
                            filler.append(f_vatp(h - 1, i, jt))
                for jt in range(njt):
                    psS = psSp.tile([P, psw], f32, tag="psS")
                    for off, sz in col_ch:
                        nc.tensor.matmul(
                            psS[:, off : off + sz],
                            kT[pb : pb + DH, hdt, jt * P : (jt + 1) * P],
                            qT[pb : pb + DH, hdt, off : off + sz],
                            start=True,
                            stop=True,
                        )
                    # drip filler to keep PE busy during the ACT-bound loop
                    budget = 2 if h == 0 else (4 if h == HPC - 1 else 3)
                    for _ in range(min(budget, len(filler))):
                        filler.pop(0)()
                    nc.scalar.activation(
                        pts[:, jt, 0:na_q],
                        psS[:, 0:na_q],
                        mybir.ActivationFunctionType.Exp,
                        bias=cmneg[:, jt : jt + 1],
                        scale=SCALE,
                    )
                    if h == HPC - 1:
                        # last head's first P@V chunk rides its own exp stream
                        filler.append(f_vatp(h, 0, jt))
            for f in filler:   # leftover filler (rare)
                f()

            psSp_cm.__exit__(None, None, None)

            # ------------- tail: last head's P@V + output projection -------
            with tc.tile_pool(name="psF", bufs=3, space="PSUM") as psFp, \
                 tc.tile_pool(name="fin", bufs=3) as fin:

                def oproj(it):
                    rows = it_sizes[it]
                    psF = psFp.tile([P, 1024], f32, tag="psF")
                    for off, sz in ((0, 512), (512, 256)):
                        for c in range(HDT):
                            nc.tensor.matmul(
                                psF[0:rows, off : off + sz],
                                attnT[:, c, it * P : it * P + rows],
                                wo_sb[:, c, off : off + sz],
                                start=(c == 0),
                                stop=(c == HDT - 1),
                            )
                    out_sb = fin.tile([P, D], f32, tag="out_sb")
                    nc.scalar.copy(out_sb[0:rows, :], psF[0:rows, 0:D])
                    # idle Pool sequencer: 25ns dispatch vs 565ns on SP
                    nc.gpsimd.dma_start(
                        out=out_d.ap()[it * P : it * P + rows, :],
                        in_=out_sb[0:rows, :],
                    )

                # both remaining P@V chunks first, then all out-proj tiles:
                # every drain chain then overlaps matmul work instead of
                # gating the final out-proj batch
                h = HPC - 1
                for i in range(1, n_qc):
                    for jt in range(njt):
                        f_vatp(h, i, jt)()
                for it in range(len(it_sizes)):
                    oproj(it)

            nrm_cm.__exit__(None, None, None)
            psxp_cm.__exit__(None, None, None)
            attn_scope.__exit__(None, None, None)

    nc.compile()
    _BUILD_CACHE[key] = nc
    return nc


def _marshal(x, x_mask, Wq, Wk, Wv, Wo):
    """Build per-core input maps. Returns (in_maps, njt, na_q, orders, counts)."""
    x = np.asarray(x, dtype=np.float32)
    x_mask = np.asarray(x_mask).astype(bool)
    Wq = np.asarray(Wq, dtype=np.float32)
    Wk = np.asarray(Wk, dtype=np.float32)
    Wv = np.asarray(Wv, dtype=np.float32)
    Wo = np.asarray(Wo, dtype=np.float32)

    orders = [np.argsort(~x_mask[b], kind="stable") for b in range(B)]
    counts = [int(x_mask[b].sum()) for b in range(B)]
    njt = max(1, -(-max(counts) // P))
    na = njt * P
    # query trimming below 512-chunk granularity fragments the PE pipeline
    # and drops the sustained PE clock (measured): keep full width
    na_q = na

    xsTs, cms = [], []
    for b in range(B):
        xs_sorted = x[b][orders[b][:na]]                 # [na, 768]
        xsTs.append(np.ascontiguousarray(xs_sorted.T.astype(BF16)))
        key_act = np.arange(na) < counts[b]
        cm = np.where(key_act, EXP_SHIFT, MASK_NEG).astype(np.float32)
        cms.append(np.ascontiguousarray(cm.reshape(njt, P).T))

    whs = []
    for hg in range(2):
        cols = slice(hg * HD, (hg + 1) * HD)
        whs.append({
            "wq": np.ascontiguousarray(Wq[:, cols].astype(BF16)),
            "wk": np.ascontiguousarray(Wk[:, cols].astype(BF16)),
            "wv": np.ascontiguousarray(Wv[:, cols].astype(BF16)),
            "woT": np.ascontiguousarray(Wo[cols, :].astype(BF16)),
        })

    in_maps = []
    for c in range(8):
        b, hg = c // 2, c % 2
        in_maps.append({
            "xsT": xsTs[b], "cmneg": cms[b], **whs[hg],
        })
    return in_maps, njt, na_q, orders, counts


def run(x, x_mask, Wq, Wk, Wv, Wo, trace=False, tmpdir=None):
    """Run on 8 cores; returns (full_output, BassKernelResults)."""
    x = np.asarray(x, dtype=np.float32)
    Wv_f = np.asarray(Wv, dtype=np.float32)
    Wo_f = np.asarray(Wo, dtype=np.float32)
    in_maps, njt, na_q, orders, counts = _marshal(x, x_mask, Wq, Wk, Wv, Wo)
    nc = build(njt, na_q)
    res = run_bass_kernel_spmd(
        nc, in_maps, core_ids=list(range(8)), trace=trace, tmpdir=tmpdir
    )
    out = np.empty((B, N, D), dtype=np.float32)
    for b in range(B):
        s = (res.results[2 * b]["out"].astype(np.float32)
             + res.results[2 * b + 1]["out"].astype(np.float32))
        nr = counts[b]
        out[b, orders[b][:nr]] = s[:nr]
        if nr < N:
            # masked queries: uniform softmax over ALL keys
            mu = x[b].astype(np.float64).mean(axis=0)
            urow = (mu @ Wv_f.astype(np.float64)) @ Wo_f.astype(np.float64)
            out[b, orders[b][nr:]] = urow.astype(np.float32)
    return out, res


def kernel(**inputs) -> np.ndarray:
    out, _ = run(
        inputs["x"], inputs["x_mask"],
        inputs["Wq"], inputs["Wk"], inputs["Wv"], inputs["Wo"],
        trace=False,
    )
    return out
